# revision 1
# baseline (speedup 1.0000x reference)
"""Trainium2 Bass kernel for nn_MinimalConvWTA_LIF.

Problem: u = stack of 3 causal FIR convs of x (taps 8/16/32), then a
sequential winner-take-all LIF scan over T=32768 steps producing binary
spikes s_all.  Outputs (u, s_all), both [B, 3, T] fp32, B=256.

Strategy (8 NeuronCores, batch-sharded 32 rows/core):
  Phase A (conv): x is loaded [quarter*32+row, t] and PE-transposed
    ([128,128] transpose covers 4 block-columns at once) into a
    [t%128, (row, zero-col + block)] layout; per 128-block piece the PE
    computes  xT_piece^T @ [W0|W1-bands]  for all 3 channels at once
    (moving operand [128, 384]), accumulating the in-block and
    previous-block band contributions in PSUM; ScalarE evacuates to
    SBUF and DMAs to the u output in DRAM (512B runs).
  Phase B (scan): the nonlinear recurrence contracts at alpha=0.95 per
    step, so time is split into 128 chunks of C=256 steps per core, each
    chunk re-simulated from zero state with a W=320-step burn-in.  All
    chunks run in lockstep: one time step = 4 VectorE ops over a
    [128, 32, 4] state tile (partition = 4 chunk-slots x 32 rows, free =
    32 chunk-groups x (3 channels + const-theta pad)):
       m    = (negmsub * -alpha) + u_t          (scalar_tensor_tensor)
       mxg  = reduce_max over (3 channels, theta)  -> spike-gated max
       s    = (m >= mxg)                        (tensor_tensor is_ge)
       negmsub = (s * theta) - m                (scalar_tensor_tensor)
    which reproduces the reference fp32 rounding bit-exactly (validated
    against lax.scan in numpy: 0/25M mismatches; burn-in 288 steps
    already gives 0 mismatches, W=320 adds margin).
    u is streamed in and s streamed out in 64-step slabs, double-buffered.
"""

import numpy as np

# ---------------------------------------------------------------------------
# Fixed problem geometry (hardcoded per contest rules)
# ---------------------------------------------------------------------------
B_FULL = 256
T_FULL = 32768
KCH = 3
N_CORES = 8
R = 32               # batch rows per core
ALPHA = np.float32(0.95)
THETA = np.float32(0.05)
TAPS = (8, 16, 32)

_prog_cache = {}


def _build_wband(w8, w16, w32):
    """Host-side: [128, 2, 3*128] fp32 banded weight matrices.

    wband[tin, 0, k*128+tau] = w_k[kk-1-(tau-tin)]       (in-block)
    wband[tin, 1, k*128+tau] = w_k[kk-1-(tau-tin+128)]   (prev-block)
    """
    ws = (np.asarray(w8, np.float32), np.asarray(w16, np.float32),
          np.asarray(w32, np.float32))
    out = np.zeros((128, 2, KCH * 128), np.float32)
    tin = np.arange(128)[:, None]
    tau = np.arange(128)[None, :]
    for k, w in enumerate(ws):
        kk = len(w)
        j0 = tau - tin           # in-block tap index
        j1 = tau - tin + 128     # prev-block tap index
        m0 = (j0 >= 0) & (j0 < kk)
        m1 = (j1 >= 0) & (j1 < kk)
        blk0 = np.zeros((128, 128), np.float32)
        blk1 = np.zeros((128, 128), np.float32)
        blk0[m0] = w[kk - 1 - j0[m0]]
        blk1[m1] = w[kk - 1 - j1[m1]]
        out[:, 0, k * 128:(k + 1) * 128] = blk0
        out[:, 1, k * 128:(k + 1) * 128] = blk1
    return out


def build_program(T=T_FULL, C=256, W=320, SLAB=64, num_devices=N_CORES,
                  do_conv=True, do_scan=True, ops_mask=15,
                  internal_outs=False, GSPL=0):
    """Build the single-core SPMD bass program.  Returns nc."""
    import concourse.bacc as bacc
    import concourse.tile as tile
    import concourse.mybir as mybir
    import concourse.bass as bass

    f32 = mybir.dt.float32
    Alu = mybir.AluOpType

    NCHUNK = T // C            # chunks per core
    assert NCHUNK % 4 == 0
    G = NCHUNK // 4            # chunk-groups along free dim
    NSTEP = C + W
    assert NSTEP % SLAB == 0 and W % SLAB == 0
    NSLAB = NSTEP // SLAB
    BURN_SLABS = W // SLAB
    NBLK = T // 128            # conv 128-blocks per row

    nc = bacc.Bacc("TRN2", target_bir_lowering=False, debug=False,
                   num_devices=num_devices)

    x_d = nc.dram_tensor("x", [R, T], f32, kind="ExternalInput")
    wb_d = nc.dram_tensor("wband", [128, 2, KCH * 128], f32,
                          kind="ExternalInput")
    id_d = nc.dram_tensor("ident", [128, 128], f32, kind="ExternalInput")
    if internal_outs:
        u_d = nc.dram_tensor("u", [R, KCH, T], f32)
        s_d = nc.dram_tensor("s", [R, KCH, T], f32)
        dummy_d = nc.dram_tensor("tinyout", [1, 4], f32, kind="ExternalOutput")
    else:
        u_d = nc.dram_tensor("u", [R, KCH, T], f32, kind="ExternalOutput")
        s_d = nc.dram_tensor("s", [R, KCH, T], f32, kind="ExternalOutput")
    x_ap = x_d.ap()
    wb_ap = wb_d.ap()
    id_ap = id_d.ap()
    u_ap = u_d.ap()
    s_ap = s_d.ap()

    with tile.TileContext(nc) as tc:
      if internal_outs:
        with tc.tile_pool(name="dumm", bufs=1) as dpool:
            dt_ = dpool.tile([1, 4], f32)
            nc.vector.memset(dt_[:, :], 1.0)
            nc.sync.dma_start(out=dummy_d.ap()[:, :], in_=dt_[:, :])
      if do_conv:
        # ================= Phase A: convolutions ======================
        BPQ = NBLK // 4      # 128-blocks per T-quarter
        with tc.tile_pool(name="xt", bufs=1) as xt_pool, \
             tc.tile_pool(name="wall", bufs=1) as w_pool, \
             tc.tile_pool(name="ustage", bufs=4) as ustage_pool, \
             tc.tile_pool(name="tpsum", bufs=4, space="PSUM") as tppool, \
             tc.tile_pool(name="cpsum", bufs=4, space="PSUM") as ppool:
            # x transposed: partition = t%128, free = (row, 1-zero-col + blocks)
            xt = xt_pool.tile([128, R, NBLK + 1], f32)
            wall = w_pool.tile([128, 2, KCH * 128], f32)
            id128 = w_pool.tile([128, 128], f32)
            nc.sync.dma_start(out=wall[:, :, :], in_=wb_ap[:, :, :])
            nc.sync.dma_start(out=id128[:, :], in_=id_ap[:, :])
            # zero xt first: gives col 0 its zeros (block -1 of the
            # prev-block matmul) and a tracked WAW dep for the fills
            nc.vector.memset(xt[:, :, :], 0.0)
            # natural x load, partition = (quarter, row)
            xq = xt_pool.tile([128, T // 4], f32)
            for q in range(4):
                nc.sync.dma_start(out=xq[q * 32:(q + 1) * 32, :],
                                  in_=x_ap[:, q * (T // 4):(q + 1) * (T // 4)])
            # one [128,128] PE transpose covers 4 xt block-columns
            # (one per quarter); ACT fans the result out into xt
            xt_t = xt[:, :, :]
            for cb in range(BPQ):
                pst = tppool.tile([128, 128], f32)
                nc.tensor.transpose(pst[:, :], xq[:, cb * 128:(cb + 1) * 128],
                                    id128[:, :])
                dst = bass.AP(xt_t.tensor, xt_t.offset + 1 + cb,
                              [list(xt_t.ap[0]), [BPQ, 4], [NBLK + 1, R]])
                nc.scalar.copy(dst, pst[:, :])

            u_blk = u_ap.rearrange("r k (b tau) -> r b k tau", tau=128)
            PIECE = min(128, NBLK)
            for r in range(R):
                for p0 in range(0, NBLK, PIECE):
                    pw = min(PIECE, NBLK - p0)
                    ps = ppool.tile([pw, KCH, 128], f32)
                    lhs0 = xt[:, r, 1 + p0: 1 + p0 + pw]
                    lhs1 = xt[:, r, p0: p0 + pw]
                    nc.tensor.matmul(ps[:, :, :], lhs0, wall[:, 0, :],
                                     start=True, stop=False)
                    nc.tensor.matmul(ps[:, :, :], lhs1, wall[:, 1, :],
                                     start=False, stop=True)
                    ust = ustage_pool.tile([pw, KCH, 128], f32)
                    nc.scalar.copy(ust[:, :, :], ps[:, :, :])
                    nc.scalar.dma_start(
                        out=u_blk[r, p0:p0 + pw, :, :],
                        in_=ust[:, :, :])

      if do_scan:
        # ================= Phase B: WTA-LIF scan ======================
        with tc.tile_pool(name="state", bufs=1) as st_pool, \
             tc.tile_pool(name="uslab", bufs=3) as upool, \
             tc.tile_pool(name="sslab", bufs=3) as spool:
            m4 = st_pool.tile([128, G, 4], f32)
            negms = st_pool.tile([128, G, KCH], f32)
            mxg = st_pool.tile([128, G], f32)
            nc.vector.memset(m4[:, :, 3], float(THETA))
            nc.vector.memset(negms[:, :, :], 0.0)

            def mxg_bcast(g_lo, g_hi):
                a = mxg[:, g_lo:g_hi]
                return bass.AP(a.tensor, a.offset, list(a.ap) + [[0, KCH]])

            for sig in range(NSLAB):
                toff = sig * SLAB - W
                ut = upool.tile([128, G, KCH, SLAB], f32)
                # chunk c = s*G + g covers t in [c*C - W, c*C + C)
                # u element for (s,r,g,k,jj): u[r, k, (s*G+g)*C + toff + jj]
                # memset regions where t < 0 (chunks 0,1 early slabs: full
                # slabs by construction since C,W,SLAB multiples of SLAB)
                g0 = 0
                while (0 * G + g0) * C + toff < 0:
                    g0 += 1          # first valid g for s=0
                if g0 > 0:
                    nc.vector.memset(ut[0:R, 0:g0, :, :], 0.0)
                # one DMA per (s-slot, channel): src dims (r, g, jj)
                for s in range(4):
                    gl = g0 if s == 0 else 0
                    if gl >= G:
                        continue
                    for k in range(KCH):
                        off = (s * G + gl) * C + toff + k * T
                        dims = [[KCH * T, R], [C, G - gl], [1, SLAB]]
                        nc.sync.dma_start(
                            out=ut[s * R:(s + 1) * R, gl:G, k, :],
                            in_=bass.AP(u_ap.tensor, off, dims))

                st = spool.tile([128, G, KCH, SLAB], f32)
                gd = GSPL  # groups [0,gd) on gpsimd, [gd,G) on vector
                for jj in range(SLAB):
                    # --- DVE chain on groups [gd, G) ---
                    nc.vector.scalar_tensor_tensor(
                        out=m4[:, gd:G, 0:KCH], in0=negms[:, gd:G, :],
                        scalar=float(-ALPHA), in1=ut[:, gd:G, :, jj],
                        op0=Alu.mult, op1=Alu.add)
                    nc.vector.tensor_reduce(
                        out=mxg[:, gd:G], in_=m4[:, gd:G, :],
                        axis=mybir.AxisListType.X, op=Alu.max)
                    nc.vector.tensor_tensor(
                        out=st[:, gd:G, :, jj], in0=m4[:, gd:G, 0:KCH],
                        in1=mxg_bcast(gd, G), op=Alu.is_ge)
                    nc.vector.scalar_tensor_tensor(
                        out=negms[:, gd:G, :], in0=st[:, gd:G, :, jj],
                        scalar=float(THETA), in1=m4[:, gd:G, 0:KCH],
                        op0=Alu.mult, op1=Alu.subtract)
                    # --- GpSimd 5-op chain on groups [0, gd) ---
                    if gd:
                        nc.gpsimd.scalar_tensor_tensor(
                            out=m4[:, 0:gd, 0:KCH], in0=negms[:, 0:gd, :],
                            scalar=float(-ALPHA), in1=ut[:, 0:gd, :, jj],
                            op0=Alu.mult, op1=Alu.add)
                        nc.gpsimd.tensor_tensor(
                            out=mxg[:, 0:gd], in0=m4[:, 0:gd, 0],
                            in1=m4[:, 0:gd, 1], op=Alu.max)
                        nc.gpsimd.scalar_tensor_tensor(
                            out=mxg[:, 0:gd], in0=m4[:, 0:gd, 2],
                            scalar=float(THETA), in1=mxg[:, 0:gd],
                            op0=Alu.max, op1=Alu.max)
                        nc.gpsimd.tensor_tensor(
                            out=st[:, 0:gd, :, jj], in0=m4[:, 0:gd, 0:KCH],
                            in1=mxg_bcast(0, gd), op=Alu.is_ge)
                        nc.gpsimd.scalar_tensor_tensor(
                            out=negms[:, 0:gd, :], in0=st[:, 0:gd, :, jj],
                            scalar=float(THETA), in1=m4[:, 0:gd, 0:KCH],
                            op0=Alu.mult, op1=Alu.subtract)

                if sig >= BURN_SLABS:
                    HS = SLAB // 2
                    for half in range(2):
                        j0 = half * HS
                        for s in range(4):
                            for k in range(KCH):
                                off = s * G * C + toff + j0 + k * T
                                dims = [[KCH * T, R], [C, G], [1, HS]]
                                nc.scalar.dma_start(
                                    out=bass.AP(s_ap.tensor, off, dims),
                                    in_=st[s * R:(s + 1) * R, :, k,
                                           j0:j0 + HS])

    nc.compile()
    return nc


def _get_program():
    key = "full"
    if key not in _prog_cache:
        _prog_cache[key] = build_program()
    return _prog_cache[key]


def _get_exec():
    """Build the 8-core PJRT callable once (mirrors run_bass_via_pjrt)."""
    if "exec" in _prog_cache:
        return _prog_cache["exec"]
    import jax
    import jax.numpy as jnp
    from jax.sharding import Mesh, PartitionSpec
    from jax.experimental.shard_map import shard_map
    import concourse.mybir as mybir
    from concourse import bass2jax

    nc = _get_program()
    bass2jax.install_neuronx_cc_hook()
    partition_name = (nc.partition_id_tensor.name
                      if nc.partition_id_tensor else None)
    in_names, out_names, out_avals, zero_shapes = [], [], [], []
    for alloc in nc.m.functions[0].allocations:
        if not isinstance(alloc, mybir.MemoryLocationSet):
            continue
        name = alloc.memorylocations[0].name
        if alloc.kind == "ExternalInput":
            if name != partition_name:
                in_names.append(name)
        elif alloc.kind == "ExternalOutput":
            out_names.append(name)
            shape = tuple(alloc.tensor_shape)
            dtype = mybir.dt.np(alloc.dtype)
            out_avals.append(jax.core.ShapedArray(shape, dtype))
            zero_shapes.append((shape, dtype))
    n_params = len(in_names)
    all_in_names = list(in_names) + list(out_names)
    if partition_name is not None:
        all_in_names.append(partition_name)

    def _body(*args):
        operands = list(args)
        if partition_name is not None:
            operands.append(bass2jax.partition_id_tensor())
        outs = bass2jax._bass_exec_p.bind(
            *operands,
            out_avals=tuple(out_avals),
            in_names=tuple(all_in_names),
            out_names=tuple(out_names),
            lowering_input_output_aliases=(),
            sim_require_finite=True,
            sim_require_nnan=True,
            nc=nc,
        )
        return tuple(outs)

    devices = jax.devices()[:N_CORES]
    assert len(devices) == N_CORES, f"need {N_CORES} devices"
    mesh = Mesh(np.asarray(devices), ("core",))
    n_outs = len(out_names)
    in_specs = (PartitionSpec("core"),) * (n_params + n_outs)
    out_specs = (PartitionSpec("core"),) * n_outs
    donate = tuple(range(n_params, n_params + n_outs))
    sharded = jax.jit(
        shard_map(_body, mesh=mesh, in_specs=in_specs, out_specs=out_specs,
                  check_rep=False),
        donate_argnums=donate, keep_unused=True)

    def make_zeros():
        return [jnp.zeros((N_CORES * s[0], *s[1:]), d)
                for (s, d) in zero_shapes]

    ex = {"nc": nc, "sharded": sharded, "in_names": in_names,
          "out_names": out_names, "make_zeros": make_zeros,
          "n_params": n_params}
    _prog_cache["exec"] = ex
    return ex


def _concat_inputs(x, w8, w16, w32):
    """Global (8*R, ...) concat inputs keyed for the program."""
    x = np.asarray(x, np.float32).reshape(B_FULL, T_FULL)
    wband = _build_wband(w8, w16, w32)
    ident = np.ascontiguousarray(np.eye(128, dtype=np.float32))
    per = {
        "x": x,                                       # already (8*R, T)
        "wband": np.concatenate([wband] * N_CORES, axis=0),
        "ident": np.concatenate([ident] * N_CORES, axis=0),
    }
    ex = _get_exec()
    return [per[name] for name in ex["in_names"]]


def kernel(x, y=None, w8=None, w16=None, w32=None):
    """Full-input entry point: x [256,1,32768], returns (u, s_all)."""
    ex = _get_exec()
    concat_in = _concat_inputs(x, w8, w16, w32)
    outs = ex["sharded"](*concat_in, *ex["make_zeros"]())
    res = {name: np.asarray(outs[i]) for i, name in enumerate(ex["out_names"])}
    u = res["u"].reshape(B_FULL, KCH, T_FULL)
    s = res["s"].reshape(B_FULL, KCH, T_FULL)
    return u, s


def bench(x, w8, w16, w32, iters=10):
    """Return list of per-call wall times (s) with device-resident I/O."""
    import time as _time
    import jax
    from jax.sharding import Mesh, PartitionSpec, NamedSharding
    ex = _get_exec()
    concat_in = _concat_inputs(x, w8, w16, w32)
    mesh = Mesh(np.asarray(jax.devices()[:N_CORES]), ("core",))
    sh = NamedSharding(mesh, PartitionSpec("core"))
    dev_in = [jax.device_put(a, sh) for a in concat_in]
    # warmup (compile)
    jax.block_until_ready(ex["sharded"](*dev_in, *ex["make_zeros"]()))
    times = []
    for _ in range(iters):
        zeros = ex["make_zeros"]()
        jax.block_until_ready(zeros)
        t0 = _time.perf_counter()
        outs = ex["sharded"](*dev_in, *zeros)
        jax.block_until_ready(outs)
        times.append(_time.perf_counter() - t0)
    return times



# revision 4
# speedup vs baseline: 1.2468x; 1.2468x over previous
"""Trainium2 Bass kernel for nn_MinimalConvWTA_LIF.

Problem: u = stack of 3 causal FIR convs of x (taps 8/16/32), then a
sequential winner-take-all LIF scan over T=32768 steps producing binary
spikes s_all.  Outputs (u, s_all), both [B, 3, T] fp32, B=256.

Strategy (8 NeuronCores, batch-sharded 32 rows/core):
  Phase A (conv): x is loaded [quarter*32+row, t] and PE-transposed into
    xt [t%128, (row, zero-col + block)].  Unlike the previous version,
    each conv matmul's OUTPUT partitions are (slot, row) = the scan
    layout: for a fixed (group g, block-half q) the moving lhs gathers,
    per output partition (s,r), the x block of chunk c = s*G+g at
    half q.  PE computes xT^T @ [W0|W1-bands] (in-block + prev-block)
    into PSUM; evac copies write u straight into the SBUF-resident
    u_ext tile in scan layout; u is DMAed to DRAM from SBUF.
  Phase B (scan): time split into chunks of C=256 steps, re-simulated
    from zero with a W-step burn-in (alpha^W contraction).  All chunks
    advance in lockstep; u is read from SBUF (no DRAM re-read).  Chunk
    burn-ins read the previous chunk's u tail via an extended group
    axis (ext group 0 = cross-slot halo filled by an SBUF DMA).  Each
    step is 4 VectorE ops over [128, G, {3,4}] tiles:
       m    = (negms * -alpha) + u_t          (scalar_tensor_tensor)
       mxg  = reduce_max over (3 ch, theta)   (tensor_reduce)
       s    = (m >= mxg)                      (tensor_tensor is_ge)
       negms = (s * theta) - m                (scalar_tensor_tensor)
    Spikes accumulate in slab tiles flushed to DRAM every SLAB steps.
"""

import numpy as np

# ---------------------------------------------------------------------------
# Fixed problem geometry (hardcoded per contest rules)
# ---------------------------------------------------------------------------
B_FULL = 256
T_FULL = 32768
KCH = 3
N_CORES = 8
R = 32               # batch rows per core
ALPHA = np.float32(0.95)
THETA = np.float32(0.05)
TAPS = (8, 16, 32)

_prog_cache = {}


def _build_wband(w8, w16, w32):
    """Host-side: [128, 2, 3*128] fp32 banded weight matrices.

    wband[tin, 0, k*128+tau] = w_k[kk-1-(tau-tin)]       (in-block)
    wband[tin, 1, k*128+tau] = w_k[kk-1-(tau-tin+128)]   (prev-block)
    """
    ws = (np.asarray(w8, np.float32), np.asarray(w16, np.float32),
          np.asarray(w32, np.float32))
    out = np.zeros((128, 2, KCH * 128), np.float32)
    tin = np.arange(128)[:, None]
    tau = np.arange(128)[None, :]
    for k, w in enumerate(ws):
        kk = len(w)
        j0 = tau - tin           # in-block tap index
        j1 = tau - tin + 128     # prev-block tap index
        m0 = (j0 >= 0) & (j0 < kk)
        m1 = (j1 >= 0) & (j1 < kk)
        blk0 = np.zeros((128, 128), np.float32)
        blk1 = np.zeros((128, 128), np.float32)
        blk0[m0] = w[kk - 1 - j0[m0]]
        blk1[m1] = w[kk - 1 - j1[m1]]
        out[:, 0, k * 128:(k + 1) * 128] = blk0
        out[:, 1, k * 128:(k + 1) * 128] = blk1
    return out


def build_program(T=T_FULL, C=256, W=160, SLAB=32, num_devices=N_CORES):
    """Build the single-core SPMD bass program.  Returns nc."""
    import concourse.bacc as bacc
    import concourse.tile as tile
    import concourse.mybir as mybir
    import concourse.bass as bass

    f32 = mybir.dt.float32
    Alu = mybir.AluOpType

    NCHUNK = T // C            # chunks per core (128 at C=256)
    assert NCHUNK % 4 == 0
    G = NCHUNK // 4            # chunk-groups along free dim (32)
    NSTEP = C + W
    assert C % SLAB == 0 and W % SLAB == 0
    NBLK = T // 128            # conv 128-blocks per row (256)
    QB = C // 128              # 128-blocks per chunk (2)
    BPS = G * QB               # blocks per slot (64)

    nc = bacc.Bacc("TRN2", target_bir_lowering=False, debug=False,
                   num_devices=num_devices)

    x_d = nc.dram_tensor("x", [R, T], f32, kind="ExternalInput")
    wb_d = nc.dram_tensor("wband", [128, 2, KCH * 128], f32,
                          kind="ExternalInput")
    id_d = nc.dram_tensor("ident", [128, 128], f32, kind="ExternalInput")
    u_d = nc.dram_tensor("u", [R, KCH, T], f32, kind="ExternalOutput")
    s_d = nc.dram_tensor("s", [R, KCH, T], f32, kind="ExternalOutput")
    x_ap = x_d.ap()
    wb_ap = wb_d.ap()
    id_ap = id_d.ap()
    u_ap = u_d.ap()
    s_ap = s_d.ap()

    with tile.TileContext(nc) as tc:
      with tc.tile_pool(name="upool", bufs=1) as u_pool:
        # u in scan layout; ext group 0 is the burn-in halo (prev chunk
        # tail); ext group 1+g holds chunk (s, g) for partition (s, r).
        u_ext = u_pool.tile([128, G + 1, KCH, C], f32)

        # ================= Phase A: convolutions ======================
        with tc.tile_pool(name="xt", bufs=1) as xt_pool, \
             tc.tile_pool(name="wall", bufs=1) as w_pool, \
             tc.tile_pool(name="tpsum", bufs=4, space="PSUM") as tppool, \
             tc.tile_pool(name="cpsum", bufs=4, space="PSUM") as ppool:
            # xt2[tin, 1+b, (s,r)] = x block (s*BPS + b) of row r.
            # col 0 = per-slot block -1 (prev slot's last block / zeros).
            xt2 = xt_pool.tile([128, BPS + 1, 128], f32)
            wall = w_pool.tile([128, 2, KCH * 128], f32)
            id128 = w_pool.tile([128, 128], f32)
            nc.sync.dma_start(out=wall[:, :, :], in_=wb_ap[:, :, :])
            nc.sync.dma_start(out=id128[:, :], in_=id_ap[:, :])
            nc.vector.memset(xt2[:, 0, :], 0.0)
            # natural x load, partition = (quarter=slot, row)
            xq = xt_pool.tile([128, T // 4], f32)
            for q in range(4):
                nc.sync.dma_start(out=xq[q * 32:(q + 1) * 32, :],
                                  in_=x_ap[:, q * (T // 4):(q + 1) * (T // 4)])
            # one [128,128] PE transpose per in-slot block: psum cols are
            # already (slot, row) order -> contiguous fan-out copy
            for cb in range(BPS):
                pst = tppool.tile([128, 128], f32)
                nc.tensor.transpose(pst[:, :], xq[:, cb * 128:(cb + 1) * 128],
                                    id128[:, :])
                nc.scalar.copy(xt2[:, 1 + cb, :], pst[:, :])
            # cross-slot halo: col 0 (s>=1) = prev slot's last block
            nc.scalar.copy(xt2[:, 0, R:128], xt2[:, BPS, 0:128 - R])

            # conv matmuls with output partition = (s, r)
            # chunk c = s*G + g covers in-slot blocks QB*g + q, q in [0, QB)
            for g in range(G):
                for q in range(QB):
                    ps = ppool.tile([128, KCH, 128], f32)
                    col = 1 + QB * g + q
                    nc.tensor.matmul(ps[:, :, :], xt2[:, col, :],
                                     wall[:, 0, :], start=True, stop=False)
                    nc.tensor.matmul(ps[:, :, :], xt2[:, col - 1, :],
                                     wall[:, 1, :], start=False, stop=True)
                    # evacuate into scan layout (alternate engines)
                    dst = u_ext[:, 1 + g, :, q * 128:(q + 1) * 128]
                    if (g * QB + q) % 2 == 0:
                        nc.scalar.copy(dst, ps[:, :, :])
                    else:
                        nc.vector.tensor_scalar(out=dst, in0=ps[:, :, :],
                                                scalar1=0.0, scalar2=None,
                                                op0=Alu.add)

            # u -> DRAM, one DMA per (slot, channel): 1KB runs
            for s in range(4):
                for k in range(KCH):
                    off = k * T + s * G * C
                    dims = [[KCH * T, R], [C, G], [1, C]]
                    nc.sync.dma_start(
                        out=bass.AP(u_ap.tensor, off, dims),
                        in_=u_ext[s * R:(s + 1) * R, 1:G + 1, k, :])

        # halo fill: ext group 0 tail = previous slot's last chunk tail
        nc.vector.memset(u_ext[0:R, 0, :, C - W:C], 0.0)
        nc.sync.dma_start(out=u_ext[R:128, 0, :, C - W:C],
                          in_=u_ext[0:128 - R, G, :, C - W:C])

        # ================= Phase B: WTA-LIF scan ======================
        with tc.tile_pool(name="state", bufs=1) as st_pool, \
             tc.tile_pool(name="sslab", bufs=3) as spool:
            m4 = st_pool.tile([128, G, 4], f32)
            negms = st_pool.tile([128, G, KCH], f32)
            mxg = st_pool.tile([128, G], f32)
            nc.vector.memset(m4[:, :, 3], float(THETA))
            nc.vector.memset(negms[:, :, :], 0.0)

            def mxg_bcast():
                a = mxg[:, :]
                return bass.AP(a.tensor, a.offset, list(a.ap) + [[0, KCH]])

            NSLAB = NSTEP // SLAB
            BURN_SLABS = W // SLAB
            for sl in range(NSLAB):
                st = spool.tile([128, G, KCH, SLAB], f32)
                for sc in range(SLAB):
                    j = sl * SLAB + sc - W       # step index in chunk frame
                    col = j if j >= 0 else C + j
                    eoff = 1 if j >= 0 else 0
                    uin = u_ext[:, eoff:eoff + G, :, col]
                    nc.vector.scalar_tensor_tensor(
                        out=m4[:, :, 0:KCH], in0=negms[:, :, :],
                        scalar=float(-ALPHA), in1=uin,
                        op0=Alu.mult, op1=Alu.add)
                    nc.vector.tensor_reduce(
                        out=mxg[:, :], in_=m4[:, :, :],
                        axis=mybir.AxisListType.X, op=Alu.max)
                    nc.vector.tensor_tensor(
                        out=st[:, :, :, sc], in0=m4[:, :, 0:KCH],
                        in1=mxg_bcast(), op=Alu.is_ge)
                    nc.vector.scalar_tensor_tensor(
                        out=negms[:, :, :], in0=st[:, :, :, sc],
                        scalar=float(THETA), in1=m4[:, :, 0:KCH],
                        op0=Alu.mult, op1=Alu.subtract)
                if sl >= BURN_SLABS:
                    j0 = sl * SLAB - W
                    for s in range(4):
                        for k in range(KCH):
                            off = s * G * C + k * T + j0
                            dims = [[KCH * T, R], [C, G], [1, SLAB]]
                            nc.scalar.dma_start(
                                out=bass.AP(s_ap.tensor, off, dims),
                                in_=st[s * R:(s + 1) * R, :, k, :])

    nc.compile()
    return nc


def _get_program():
    key = "full"
    if key not in _prog_cache:
        _prog_cache[key] = build_program()
    return _prog_cache[key]


def _get_exec():
    """Build the 8-core PJRT callable once (mirrors run_bass_via_pjrt)."""
    if "exec" in _prog_cache:
        return _prog_cache["exec"]
    import jax
    import jax.numpy as jnp
    from jax.sharding import Mesh, PartitionSpec
    from jax.experimental.shard_map import shard_map
    import concourse.mybir as mybir
    from concourse import bass2jax

    nc = _get_program()
    bass2jax.install_neuronx_cc_hook()
    partition_name = (nc.partition_id_tensor.name
                      if nc.partition_id_tensor else None)
    in_names, out_names, out_avals, zero_shapes = [], [], [], []
    for alloc in nc.m.functions[0].allocations:
        if not isinstance(alloc, mybir.MemoryLocationSet):
            continue
        name = alloc.memorylocations[0].name
        if alloc.kind == "ExternalInput":
            if name != partition_name:
                in_names.append(name)
        elif alloc.kind == "ExternalOutput":
            out_names.append(name)
            shape = tuple(alloc.tensor_shape)
            dtype = mybir.dt.np(alloc.dtype)
            out_avals.append(jax.core.ShapedArray(shape, dtype))
            zero_shapes.append((shape, dtype))
    n_params = len(in_names)
    all_in_names = list(in_names) + list(out_names)
    if partition_name is not None:
        all_in_names.append(partition_name)

    def _body(*args):
        operands = list(args)
        if partition_name is not None:
            operands.append(bass2jax.partition_id_tensor())
        outs = bass2jax._bass_exec_p.bind(
            *operands,
            out_avals=tuple(out_avals),
            in_names=tuple(all_in_names),
            out_names=tuple(out_names),
            lowering_input_output_aliases=(),
            sim_require_finite=True,
            sim_require_nnan=True,
            nc=nc,
        )
        return tuple(outs)

    devices = jax.devices()[:N_CORES]
    assert len(devices) == N_CORES, f"need {N_CORES} devices"
    mesh = Mesh(np.asarray(devices), ("core",))
    n_outs = len(out_names)
    in_specs = (PartitionSpec("core"),) * (n_params + n_outs)
    out_specs = (PartitionSpec("core"),) * n_outs
    donate = tuple(range(n_params, n_params + n_outs))
    sharded = jax.jit(
        shard_map(_body, mesh=mesh, in_specs=in_specs, out_specs=out_specs,
                  check_rep=False),
        donate_argnums=donate, keep_unused=True)

    def make_zeros():
        return [jnp.zeros((N_CORES * s[0], *s[1:]), d)
                for (s, d) in zero_shapes]

    ex = {"nc": nc, "sharded": sharded, "in_names": in_names,
          "out_names": out_names, "make_zeros": make_zeros,
          "n_params": n_params}
    _prog_cache["exec"] = ex
    return ex


def _concat_inputs(x, w8, w16, w32):
    """Global (8*R, ...) concat inputs keyed for the program."""
    x = np.asarray(x, np.float32).reshape(B_FULL, T_FULL)
    wband = _build_wband(w8, w16, w32)
    ident = np.ascontiguousarray(np.eye(128, dtype=np.float32))
    per = {
        "x": x,                                       # already (8*R, T)
        "wband": np.concatenate([wband] * N_CORES, axis=0),
        "ident": np.concatenate([ident] * N_CORES, axis=0),
    }
    ex = _get_exec()
    return [per[name] for name in ex["in_names"]]


def kernel(x, y=None, w8=None, w16=None, w32=None):
    """Full-input entry point: x [256,1,32768], returns (u, s_all)."""
    ex = _get_exec()
    concat_in = _concat_inputs(x, w8, w16, w32)
    outs = ex["sharded"](*concat_in, *ex["make_zeros"]())
    res = {name: np.asarray(outs[i]) for i, name in enumerate(ex["out_names"])}
    u = res["u"].reshape(B_FULL, KCH, T_FULL)
    s = res["s"].reshape(B_FULL, KCH, T_FULL)
    return u, s


def bench(x, w8, w16, w32, iters=10):
    """Return list of per-call wall times (s) with device-resident I/O."""
    import time as _time
    import jax
    from jax.sharding import Mesh, PartitionSpec, NamedSharding
    ex = _get_exec()
    concat_in = _concat_inputs(x, w8, w16, w32)
    mesh = Mesh(np.asarray(jax.devices()[:N_CORES]), ("core",))
    sh = NamedSharding(mesh, PartitionSpec("core"))
    dev_in = [jax.device_put(a, sh) for a in concat_in]
    jax.block_until_ready(ex["sharded"](*dev_in, *ex["make_zeros"]()))
    times = []
    for _ in range(iters):
        zeros = ex["make_zeros"]()
        jax.block_until_ready(zeros)
        t0 = _time.perf_counter()
        outs = ex["sharded"](*dev_in, *zeros)
        jax.block_until_ready(outs)
        times.append(_time.perf_counter() - t0)
    return times


# revision 8
# speedup vs baseline: 1.3705x; 1.0992x over previous
"""Trainium2 Bass kernel for nn_MinimalConvWTA_LIF.

Problem: u = stack of 3 causal FIR convs of x (taps 8/16/32), then a
sequential winner-take-all LIF scan over T=32768 steps producing binary
spikes s_all.  Outputs (u, s_all), both [B, 3, T] fp32, B=256.

Strategy (8 NeuronCores, batch-sharded 32 rows/core):
  Phase A (conv): x is loaded [quarter*32+row, t] and PE-transposed into
    xt [t%128, (row, zero-col + block)].  Unlike the previous version,
    each conv matmul's OUTPUT partitions are (slot, row) = the scan
    layout: for a fixed (group g, block-half q) the moving lhs gathers,
    per output partition (s,r), the x block of chunk c = s*G+g at
    half q.  PE computes xT^T @ [W0|W1-bands] (in-block + prev-block)
    into PSUM; evac copies write u straight into the SBUF-resident
    u_ext tile in scan layout; u is DMAed to DRAM from SBUF.
  Phase B (scan): time split into chunks of C=256 steps, re-simulated
    from zero with a W-step burn-in (alpha^W contraction).  All chunks
    advance in lockstep; u is read from SBUF (no DRAM re-read).  Chunk
    burn-ins read the previous chunk's u tail via an extended group
    axis (ext group 0 = cross-slot halo filled by an SBUF DMA).  Each
    step is 4 VectorE ops over [128, G, {3,4}] tiles:
       m    = (negms * -alpha) + u_t          (scalar_tensor_tensor)
       mxg  = reduce_max over (3 ch, theta)   (tensor_reduce)
       s    = (m >= mxg)                      (tensor_tensor is_ge)
       negms = (s * theta) - m                (scalar_tensor_tensor)
    Spikes accumulate in slab tiles flushed to DRAM every SLAB steps.
"""

import numpy as np

# ---------------------------------------------------------------------------
# Fixed problem geometry (hardcoded per contest rules)
# ---------------------------------------------------------------------------
B_FULL = 256
T_FULL = 32768
KCH = 3
N_CORES = 8
R = 32               # batch rows per core
ALPHA = np.float32(0.95)
THETA = np.float32(0.05)
TAPS = (8, 16, 32)

_prog_cache = {}


def _build_wband(w8, w16, w32):
    """Host-side: [128, 2, 3*128] fp32 banded weight matrices.

    wband[tin, 0, k*128+tau] = w_k[kk-1-(tau-tin)]       (in-block)
    wband[tin, 1, k*128+tau] = w_k[kk-1-(tau-tin+128)]   (prev-block)
    """
    ws = (np.asarray(w8, np.float32), np.asarray(w16, np.float32),
          np.asarray(w32, np.float32))
    out = np.zeros((128, 2, KCH * 128), np.float32)
    tin = np.arange(128)[:, None]
    tau = np.arange(128)[None, :]
    for k, w in enumerate(ws):
        kk = len(w)
        j0 = tau - tin           # in-block tap index
        j1 = tau - tin + 128     # prev-block tap index
        m0 = (j0 >= 0) & (j0 < kk)
        m1 = (j1 >= 0) & (j1 < kk)
        blk0 = np.zeros((128, 128), np.float32)
        blk1 = np.zeros((128, 128), np.float32)
        blk0[m0] = w[kk - 1 - j0[m0]]
        blk1[m1] = w[kk - 1 - j1[m1]]
        out[:, 0, k * 128:(k + 1) * 128] = blk0
        out[:, 1, k * 128:(k + 1) * 128] = blk1
    return out


def build_program(T=T_FULL, C=256, W=160, SLAB=32, ILEAVE=2,
                  num_devices=N_CORES):
    """Build the single-core SPMD bass program.  Returns nc."""
    import concourse.bacc as bacc
    import concourse.tile as tile
    import concourse.mybir as mybir
    import concourse.bass as bass

    f32 = mybir.dt.float32
    Alu = mybir.AluOpType

    NCHUNK = T // C            # chunks per core (128 at C=256)
    assert NCHUNK % 4 == 0
    G = NCHUNK // 4            # chunk-groups along free dim (32)
    NSTEP = C + W
    assert C % SLAB == 0 and W % SLAB == 0
    NBLK = T // 128            # conv 128-blocks per row (256)
    QB = C // 128              # 128-blocks per chunk (2)
    BPS = G * QB               # blocks per slot (64)

    nc = bacc.Bacc("TRN2", target_bir_lowering=False, debug=False,
                   num_devices=num_devices)

    x_d = nc.dram_tensor("x", [R, T], f32, kind="ExternalInput")
    wb_d = nc.dram_tensor("wband", [128, 2, KCH * 128], f32,
                          kind="ExternalInput")
    id_d = nc.dram_tensor("ident", [128, 128], f32, kind="ExternalInput")
    u_d = nc.dram_tensor("u", [R, KCH, T], f32, kind="ExternalOutput")
    s_d = nc.dram_tensor("s", [R, KCH, T], f32, kind="ExternalOutput")
    x_ap = x_d.ap()
    wb_ap = wb_d.ap()
    id_ap = id_d.ap()
    u_ap = u_d.ap()
    s_ap = s_d.ap()

    with tile.TileContext(nc) as tc:
      with tc.tile_pool(name="upool", bufs=1) as u_pool:
        # u in scan layout; ext group 0 is the burn-in halo (prev chunk
        # tail); ext group 1+g holds chunk (s, g) for partition (s, r).
        u_ext = u_pool.tile([128, G + 1, KCH, C], f32)

        # ================= Phase A: convolutions ======================
        with tc.tile_pool(name="xt", bufs=1) as xt_pool, \
             tc.tile_pool(name="wall", bufs=1) as w_pool:
            # xt2[tin, 1+b, (s,r)] = x block (s*BPS + b) of row r.
            # col 0 = per-slot block -1 (prev slot's last block / zeros).
            xt2 = xt_pool.tile([128, BPS + 1, 128], f32)
            wall = w_pool.tile([128, 2, KCH * 128], f32)
            id128 = w_pool.tile([128, 128], f32)
            nc.sync.dma_start(out=wall[:, :, :], in_=wb_ap[:, :, :])
            nc.sync.dma_start(out=id128[:, :], in_=id_ap[:, :])
            nc.vector.memset(xt2[:, 0, :], 0.0)
            # natural x load, partition = (quarter=slot, row)
            xq = xt_pool.tile([128, T // 4], f32)
            for q in range(4):
                for h in range(2):
                    nc.sync.dma_start(
                        out=xq[q * 32 + h * 16:q * 32 + (h + 1) * 16, :],
                        in_=x_ap[h * 16:(h + 1) * 16,
                                 q * (T // 4):(q + 1) * (T // 4)])
            # one [128,128] PE transpose per in-slot block: psum cols are
            # already (slot, row) order -> contiguous fan-out copy
            with tc.tile_pool(name="tpsum", bufs=8, space="PSUM") as tppool:
                for cb in range(BPS):
                    pst = tppool.tile([128, 128], f32)
                    nc.tensor.transpose(pst[:, :],
                                        xq[:, cb * 128:(cb + 1) * 128],
                                        id128[:, :])
                    if cb % 2 == 0:
                        nc.scalar.copy(xt2[:, 1 + cb, :], pst[:, :])
                    else:
                        nc.vector.tensor_scalar(out=xt2[:, 1 + cb, :],
                                                in0=pst[:, :], scalar1=0.0,
                                                scalar2=None, op0=Alu.add)
            # cross-slot halo: col 0 (s>=1) = prev slot's last block
            nc.scalar.copy(xt2[:, 0, R:128], xt2[:, BPS, 0:128 - R])

            # conv matmuls with output partition = (s, r); prev-block band
            # only has taps in tin >= 97, so contract over 31 rows
            with tc.tile_pool(name="cpsum", bufs=8, space="PSUM") as ppool:
                for g in range(G):
                    for q in range(QB):
                        ps = ppool.tile([128, KCH, 128], f32)
                        col = 1 + QB * g + q
                        nc.tensor.matmul(ps[:, :, :], xt2[:, col, :],
                                         wall[:, 0, :], start=True, stop=False)
                        nc.tensor.matmul(ps[:, :, :], xt2[64:128, col - 1, :],
                                         wall[64:128, 1, :],
                                         start=False, stop=True)
                        # evacuate into scan layout (alternate engines)
                        dst = u_ext[:, 1 + g, :, q * 128:(q + 1) * 128]
                        if (g * QB + q) % 2 == 0:
                            nc.scalar.copy(dst, ps[:, :, :])
                        else:
                            nc.vector.tensor_scalar(out=dst, in0=ps[:, :, :],
                                                    scalar1=0.0, scalar2=None,
                                                    op0=Alu.add)

            # u -> DRAM, one DMA per (slot, channel): 1KB runs
            for s in range(4):
                for k in range(KCH):
                    off = k * T + s * G * C
                    dims = [[KCH * T, R], [C, G], [1, C]]
                    nc.sync.dma_start(
                        out=bass.AP(u_ap.tensor, off, dims),
                        in_=u_ext[s * R:(s + 1) * R, 1:G + 1, k, :])

        # halo fill: ext group 0 tail = previous slot's last chunk tail
        nc.vector.memset(u_ext[0:R, 0, :, C - W:C], 0.0)
        nc.sync.dma_start(out=u_ext[R:128, 0, :, C - W:C],
                          in_=u_ext[0:128 - R, G, :, C - W:C])

        # ================= Phase B: WTA-LIF scan ======================
        with tc.tile_pool(name="state", bufs=1) as st_pool, \
             tc.tile_pool(name="sslab", bufs=3) as spool:
            m4 = st_pool.tile([128, G, 4], f32)
            negms = st_pool.tile([128, G, KCH], f32)
            mxg = st_pool.tile([128, G], f32)
            nc.vector.memset(m4[:, :, 3], float(THETA))
            nc.vector.memset(negms[:, :, :], 0.0)

            def mxg_bcast(lo, hi):
                a = mxg[:, lo:hi]
                return bass.AP(a.tensor, a.offset, list(a.ap) + [[0, KCH]])

            assert G % ILEAVE == 0
            GH = G // ILEAVE
            parts = [(h * GH, (h + 1) * GH) for h in range(ILEAVE)]

            NSLAB = NSTEP // SLAB
            BURN_SLABS = W // SLAB
            for sl in range(NSLAB):
                st = spool.tile([128, G, KCH, SLAB], f32)
                for sc in range(SLAB):
                    j = sl * SLAB + sc - W       # step index in chunk frame
                    col = j if j >= 0 else C + j
                    eoff = 1 if j >= 0 else 0
                    for lo, hi in parts:
                        uin = u_ext[:, eoff + lo:eoff + hi, :, col]
                        nc.vector.scalar_tensor_tensor(
                            out=m4[:, lo:hi, 0:KCH], in0=negms[:, lo:hi, :],
                            scalar=float(-ALPHA), in1=uin,
                            op0=Alu.mult, op1=Alu.add)
                    for lo, hi in parts:
                        nc.vector.tensor_reduce(
                            out=mxg[:, lo:hi], in_=m4[:, lo:hi, :],
                            axis=mybir.AxisListType.X, op=Alu.max)
                    for lo, hi in parts:
                        nc.vector.tensor_tensor(
                            out=st[:, lo:hi, :, sc], in0=m4[:, lo:hi, 0:KCH],
                            in1=mxg_bcast(lo, hi), op=Alu.is_ge)
                    for lo, hi in parts:
                        nc.vector.scalar_tensor_tensor(
                            out=negms[:, lo:hi, :], in0=st[:, lo:hi, :, sc],
                            scalar=float(THETA), in1=m4[:, lo:hi, 0:KCH],
                            op0=Alu.mult, op1=Alu.subtract)
                if sl >= BURN_SLABS:
                    j0 = sl * SLAB - W
                    for s in range(4):
                        for k in range(KCH):
                            off = s * G * C + k * T + j0
                            dims = [[KCH * T, R], [C, G], [1, SLAB]]
                            eng = nc.scalar if (s * KCH + k) % 2 == 0 else nc.sync
                            eng.dma_start(
                                out=bass.AP(s_ap.tensor, off, dims),
                                in_=st[s * R:(s + 1) * R, :, k, :])

    nc.compile()
    return nc


def _get_program():
    key = "full"
    if key not in _prog_cache:
        _prog_cache[key] = build_program()
    return _prog_cache[key]


def _get_exec():
    """Build the 8-core PJRT callable once (mirrors run_bass_via_pjrt)."""
    if "exec" in _prog_cache:
        return _prog_cache["exec"]
    import jax
    import jax.numpy as jnp
    from jax.sharding import Mesh, PartitionSpec
    from jax.experimental.shard_map import shard_map
    import concourse.mybir as mybir
    from concourse import bass2jax

    nc = _get_program()
    bass2jax.install_neuronx_cc_hook()
    partition_name = (nc.partition_id_tensor.name
                      if nc.partition_id_tensor else None)
    in_names, out_names, out_avals, zero_shapes = [], [], [], []
    for alloc in nc.m.functions[0].allocations:
        if not isinstance(alloc, mybir.MemoryLocationSet):
            continue
        name = alloc.memorylocations[0].name
        if alloc.kind == "ExternalInput":
            if name != partition_name:
                in_names.append(name)
        elif alloc.kind == "ExternalOutput":
            out_names.append(name)
            shape = tuple(alloc.tensor_shape)
            dtype = mybir.dt.np(alloc.dtype)
            out_avals.append(jax.core.ShapedArray(shape, dtype))
            zero_shapes.append((shape, dtype))
    n_params = len(in_names)
    all_in_names = list(in_names) + list(out_names)
    if partition_name is not None:
        all_in_names.append(partition_name)

    def _body(*args):
        operands = list(args)
        if partition_name is not None:
            operands.append(bass2jax.partition_id_tensor())
        outs = bass2jax._bass_exec_p.bind(
            *operands,
            out_avals=tuple(out_avals),
            in_names=tuple(all_in_names),
            out_names=tuple(out_names),
            lowering_input_output_aliases=(),
            sim_require_finite=True,
            sim_require_nnan=True,
            nc=nc,
        )
        return tuple(outs)

    devices = jax.devices()[:N_CORES]
    assert len(devices) == N_CORES, f"need {N_CORES} devices"
    mesh = Mesh(np.asarray(devices), ("core",))
    n_outs = len(out_names)
    in_specs = (PartitionSpec("core"),) * (n_params + n_outs)
    out_specs = (PartitionSpec("core"),) * n_outs
    donate = tuple(range(n_params, n_params + n_outs))
    sharded = jax.jit(
        shard_map(_body, mesh=mesh, in_specs=in_specs, out_specs=out_specs,
                  check_rep=False),
        donate_argnums=donate, keep_unused=True)

    def make_zeros():
        return [jnp.zeros((N_CORES * s[0], *s[1:]), d)
                for (s, d) in zero_shapes]

    ex = {"nc": nc, "sharded": sharded, "in_names": in_names,
          "out_names": out_names, "make_zeros": make_zeros,
          "n_params": n_params}
    _prog_cache["exec"] = ex
    return ex


def _concat_inputs(x, w8, w16, w32):
    """Global (8*R, ...) concat inputs keyed for the program."""
    x = np.asarray(x, np.float32).reshape(B_FULL, T_FULL)
    wband = _build_wband(w8, w16, w32)
    ident = np.ascontiguousarray(np.eye(128, dtype=np.float32))
    per = {
        "x": x,                                       # already (8*R, T)
        "wband": np.concatenate([wband] * N_CORES, axis=0),
        "ident": np.concatenate([ident] * N_CORES, axis=0),
    }
    ex = _get_exec()
    return [per[name] for name in ex["in_names"]]


def kernel(x, y=None, w8=None, w16=None, w32=None):
    """Full-input entry point: x [256,1,32768], returns (u, s_all)."""
    ex = _get_exec()
    concat_in = _concat_inputs(x, w8, w16, w32)
    outs = ex["sharded"](*concat_in, *ex["make_zeros"]())
    res = {name: np.asarray(outs[i]) for i, name in enumerate(ex["out_names"])}
    u = res["u"].reshape(B_FULL, KCH, T_FULL)
    s = res["s"].reshape(B_FULL, KCH, T_FULL)
    return u, s


def bench(x, w8, w16, w32, iters=10):
    """Return list of per-call wall times (s) with device-resident I/O."""
    import time as _time
    import jax
    from jax.sharding import Mesh, PartitionSpec, NamedSharding
    ex = _get_exec()
    concat_in = _concat_inputs(x, w8, w16, w32)
    mesh = Mesh(np.asarray(jax.devices()[:N_CORES]), ("core",))
    sh = NamedSharding(mesh, PartitionSpec("core"))
    dev_in = [jax.device_put(a, sh) for a in concat_in]
    jax.block_until_ready(ex["sharded"](*dev_in, *ex["make_zeros"]()))
    times = []
    for _ in range(iters):
        zeros = ex["make_zeros"]()
        jax.block_until_ready(zeros)
        t0 = _time.perf_counter()
        outs = ex["sharded"](*dev_in, *zeros)
        jax.block_until_ready(outs)
        times.append(_time.perf_counter() - t0)
    return times


# revision 9
# speedup vs baseline: 1.6515x; 1.2050x over previous
"""Trainium2 Bass kernel for nn_MinimalConvWTA_LIF.

Problem: u = stack of 3 causal FIR convs of x (taps 8/16/32), then a
sequential winner-take-all LIF scan over T=32768 steps producing binary
spikes s_all.  Outputs (u, s_all), both [B, 3, T] fp32, B=256.

Strategy (8 NeuronCores, batch-sharded 32 rows/core):
  Phase A (conv): x is loaded [quarter*32+row, t] and PE-transposed into
    xt [t%128, (row, zero-col + block)].  Unlike the previous version,
    each conv matmul's OUTPUT partitions are (slot, row) = the scan
    layout: for a fixed (group g, block-half q) the moving lhs gathers,
    per output partition (s,r), the x block of chunk c = s*G+g at
    half q.  PE computes xT^T @ [W0|W1-bands] (in-block + prev-block)
    into PSUM; evac copies write u straight into the SBUF-resident
    u_ext tile in scan layout; u is DMAed to DRAM from SBUF.
  Phase B (scan): time split into chunks of C=256 steps, re-simulated
    from zero with a W-step burn-in (alpha^W contraction).  All chunks
    advance in lockstep; u is read from SBUF (no DRAM re-read).  Chunk
    burn-ins read the previous chunk's u tail via an extended group
    axis (ext group 0 = cross-slot halo filled by an SBUF DMA).  Each
    step is 4 VectorE ops over [128, G, {3,4}] tiles:
       m    = (negms * -alpha) + u_t          (scalar_tensor_tensor)
       mxg  = reduce_max over (3 ch, theta)   (tensor_reduce)
       s    = (m >= mxg)                      (tensor_tensor is_ge)
       negms = (s * theta) - m                (scalar_tensor_tensor)
    Spikes accumulate in slab tiles flushed to DRAM every SLAB steps.
"""

import numpy as np

# ---------------------------------------------------------------------------
# Fixed problem geometry (hardcoded per contest rules)
# ---------------------------------------------------------------------------
B_FULL = 256
T_FULL = 32768
KCH = 3
N_CORES = 8
R = 32               # batch rows per core
ALPHA = np.float32(0.95)
THETA = np.float32(0.05)
TAPS = (8, 16, 32)

_prog_cache = {}


def _build_wband(w8, w16, w32):
    """Host-side: [128, 2, 3*128] fp32 banded weight matrices.

    wband[tin, 0, k*128+tau] = w_k[kk-1-(tau-tin)]       (in-block)
    wband[tin, 1, k*128+tau] = w_k[kk-1-(tau-tin+128)]   (prev-block)
    """
    ws = (np.asarray(w8, np.float32), np.asarray(w16, np.float32),
          np.asarray(w32, np.float32))
    out = np.zeros((128, 2, KCH * 128), np.float32)
    tin = np.arange(128)[:, None]
    tau = np.arange(128)[None, :]
    for k, w in enumerate(ws):
        kk = len(w)
        j0 = tau - tin           # in-block tap index
        j1 = tau - tin + 128     # prev-block tap index
        m0 = (j0 >= 0) & (j0 < kk)
        m1 = (j1 >= 0) & (j1 < kk)
        blk0 = np.zeros((128, 128), np.float32)
        blk1 = np.zeros((128, 128), np.float32)
        blk0[m0] = w[kk - 1 - j0[m0]]
        blk1[m1] = w[kk - 1 - j1[m1]]
        out[:, 0, k * 128:(k + 1) * 128] = blk0
        out[:, 1, k * 128:(k + 1) * 128] = blk1
    return out


def build_program(T=T_FULL, C=256, W=128, SLAB=32, ILEAVE=2,
                  num_devices=N_CORES):
    """Build the single-core SPMD bass program.  Returns nc."""
    import concourse.bacc as bacc
    import concourse.tile as tile
    import concourse.mybir as mybir
    import concourse.bass as bass

    f32 = mybir.dt.float32
    Alu = mybir.AluOpType

    NCHUNK = T // C            # chunks per core (128 at C=256)
    assert NCHUNK % 4 == 0
    G = NCHUNK // 4            # chunk-groups along free dim (32)
    NSTEP = C + W
    assert C % SLAB == 0 and W % SLAB == 0
    NBLK = T // 128            # conv 128-blocks per row (256)
    QB = C // 128              # 128-blocks per chunk (2)
    BPS = G * QB               # blocks per slot (64)

    nc = bacc.Bacc("TRN2", target_bir_lowering=False, debug=False,
                   num_devices=num_devices)

    x_d = nc.dram_tensor("x", [R, T], f32, kind="ExternalInput")
    wb_d = nc.dram_tensor("wband", [128, 2, KCH * 128], f32,
                          kind="ExternalInput")
    id_d = nc.dram_tensor("ident", [128, 128], f32, kind="ExternalInput")
    u_d = nc.dram_tensor("u", [R, KCH, T], f32, kind="ExternalOutput")
    s_d = nc.dram_tensor("s", [R, KCH, T], f32, kind="ExternalOutput")
    x_ap = x_d.ap()
    wb_ap = wb_d.ap()
    id_ap = id_d.ap()
    u_ap = u_d.ap()
    s_ap = s_d.ap()

    with tile.TileContext(nc) as tc:
      with tc.tile_pool(name="upool", bufs=1) as u_pool:
        # u in scan layout; ext group 0 is the burn-in halo (prev chunk
        # tail); ext group 1+g holds chunk (s, g) for partition (s, r).
        u_ext = u_pool.tile([128, G + 1, KCH, C], f32)

        # ================= Phase A: convolutions ======================
        with tc.tile_pool(name="xt", bufs=1) as xt_pool, \
             tc.tile_pool(name="wall", bufs=1) as w_pool:
            # xt2[tin, 1+b, (s,r)] = x block (s*BPS + b) of row r.
            # col 0 = per-slot block -1 (prev slot's last block / zeros).
            xt2 = xt_pool.tile([128, BPS + 1, 128], f32)
            wall = w_pool.tile([128, 2, KCH * 128], f32)
            id128 = w_pool.tile([128, 128], f32)
            nc.sync.dma_start(out=wall[:, :, :], in_=wb_ap[:, :, :])
            nc.sync.dma_start(out=id128[:, :], in_=id_ap[:, :])
            nc.vector.memset(xt2[:, 0, :], 0.0)
            # natural x load, partition = (quarter=slot, row)
            xq = xt_pool.tile([128, T // 4], f32)
            SEG = T // 16
            for seg in range(4):
                for q in range(4):
                    nc.sync.dma_start(
                        out=xq[q * 32:(q + 1) * 32,
                               seg * SEG:(seg + 1) * SEG],
                        in_=x_ap[:, q * (T // 4) + seg * SEG:
                                 q * (T // 4) + (seg + 1) * SEG])
            # one [128,128] PE transpose per in-slot block: psum cols are
            # already (slot, row) order -> contiguous fan-out copy
            with tc.tile_pool(name="tpsum", bufs=8, space="PSUM") as tppool:
                for cb in range(BPS):
                    pst = tppool.tile([128, 128], f32)
                    nc.tensor.transpose(pst[:, :],
                                        xq[:, cb * 128:(cb + 1) * 128],
                                        id128[:, :])
                    if cb % 2 == 0:
                        nc.scalar.copy(xt2[:, 1 + cb, :], pst[:, :])
                    else:
                        nc.vector.tensor_scalar(out=xt2[:, 1 + cb, :],
                                                in0=pst[:, :], scalar1=0.0,
                                                scalar2=None, op0=Alu.add)
            # cross-slot halo: col 0 (s>=1) = prev slot's last block
            nc.scalar.copy(xt2[:, 0, R:128], xt2[:, BPS, 0:128 - R])

            # conv matmuls with output partition = (s, r); prev-block band
            # only has taps in tin >= 97 -> contract rows 64:128.
            # Emission order: q=1 for g=31 then g=0..30 (burn-in consumes
            # q=1 columns first and the halo needs g=31), then q=0.
            order = []
            for q in (1, 0):
                for g in [G - 1] + list(range(G - 1)):
                    order.append((g, q))
            with tc.tile_pool(name="cpsum", bufs=8, space="PSUM") as ppool:
                for idx, (g, q) in enumerate(order):
                    ps = ppool.tile([128, KCH, 128], f32)
                    col = 1 + QB * g + q
                    nc.tensor.matmul(ps[:, :, :], xt2[:, col, :],
                                     wall[:, 0, :], start=True, stop=False)
                    nc.tensor.matmul(ps[:, :, :], xt2[64:128, col - 1, :],
                                     wall[64:128, 1, :],
                                     start=False, stop=True)
                    # evacuate into scan layout (alternate engines)
                    dst = u_ext[:, 1 + g, :, q * 128:(q + 1) * 128]
                    if idx % 2 == 0:
                        nc.scalar.copy(dst, ps[:, :, :])
                    else:
                        nc.vector.tensor_scalar(out=dst, in0=ps[:, :, :],
                                                scalar1=0.0, scalar2=None,
                                                op0=Alu.add)

            # u -> DRAM, one DMA per (slot, channel): 1KB runs
            for s in range(4):
                for k in range(KCH):
                    off = k * T + s * G * C
                    dims = [[KCH * T, R], [C, G], [1, C]]
                    nc.sync.dma_start(
                        out=bass.AP(u_ap.tensor, off, dims),
                        in_=u_ext[s * R:(s + 1) * R, 1:G + 1, k, :])

        # halo fill: ext group 0 tail = previous slot's last chunk tail
        nc.vector.memset(u_ext[0:R, 0, :, C - W:C], 0.0)
        nc.sync.dma_start(out=u_ext[R:128, 0, :, C - W:C],
                          in_=u_ext[0:128 - R, G, :, C - W:C])

        # ================= Phase B: WTA-LIF scan ======================
        with tc.tile_pool(name="state", bufs=1) as st_pool, \
             tc.tile_pool(name="sslab", bufs=3) as spool:
            m4 = st_pool.tile([128, G, 4], f32)
            negms = st_pool.tile([128, G, KCH], f32)
            mxg = st_pool.tile([128, G], f32)
            nc.vector.memset(m4[:, :, 3], float(THETA))
            nc.vector.memset(negms[:, :, :], 0.0)

            def mxg_bcast(lo, hi):
                a = mxg[:, lo:hi]
                return bass.AP(a.tensor, a.offset, list(a.ap) + [[0, KCH]])

            assert G % ILEAVE == 0
            GH = G // ILEAVE
            parts = [(h * GH, (h + 1) * GH) for h in range(ILEAVE)]

            NSLAB = NSTEP // SLAB
            BURN_SLABS = W // SLAB
            for sl in range(NSLAB):
                st = spool.tile([128, G, KCH, SLAB], f32)
                for sc in range(SLAB):
                    j = sl * SLAB + sc - W       # step index in chunk frame
                    col = j if j >= 0 else C + j
                    eoff = 1 if j >= 0 else 0
                    for lo, hi in parts:
                        uin = u_ext[:, eoff + lo:eoff + hi, :, col]
                        nc.vector.scalar_tensor_tensor(
                            out=m4[:, lo:hi, 0:KCH], in0=negms[:, lo:hi, :],
                            scalar=float(-ALPHA), in1=uin,
                            op0=Alu.mult, op1=Alu.add)
                    for lo, hi in parts:
                        nc.vector.tensor_reduce(
                            out=mxg[:, lo:hi], in_=m4[:, lo:hi, :],
                            axis=mybir.AxisListType.X, op=Alu.max)
                    for lo, hi in parts:
                        nc.vector.tensor_tensor(
                            out=st[:, lo:hi, :, sc], in0=m4[:, lo:hi, 0:KCH],
                            in1=mxg_bcast(lo, hi), op=Alu.is_ge)
                    for lo, hi in parts:
                        nc.vector.scalar_tensor_tensor(
                            out=negms[:, lo:hi, :], in0=st[:, lo:hi, :, sc],
                            scalar=float(THETA), in1=m4[:, lo:hi, 0:KCH],
                            op0=Alu.mult, op1=Alu.subtract)
                if sl >= BURN_SLABS:
                    j0 = sl * SLAB - W
                    for s in range(4):
                        for k in range(KCH):
                            off = s * G * C + k * T + j0
                            dims = [[KCH * T, R], [C, G], [1, SLAB]]
                            eng = nc.scalar if (s * KCH + k) % 2 == 0 else nc.sync
                            eng.dma_start(
                                out=bass.AP(s_ap.tensor, off, dims),
                                in_=st[s * R:(s + 1) * R, :, k, :])

    nc.compile()
    return nc


def _get_program():
    key = "full"
    if key not in _prog_cache:
        _prog_cache[key] = build_program()
    return _prog_cache[key]


def _get_exec():
    """Build the 8-core PJRT callable once (mirrors run_bass_via_pjrt)."""
    if "exec" in _prog_cache:
        return _prog_cache["exec"]
    import jax
    import jax.numpy as jnp
    from jax.sharding import Mesh, PartitionSpec
    from jax.experimental.shard_map import shard_map
    import concourse.mybir as mybir
    from concourse import bass2jax

    nc = _get_program()
    bass2jax.install_neuronx_cc_hook()
    partition_name = (nc.partition_id_tensor.name
                      if nc.partition_id_tensor else None)
    in_names, out_names, out_avals, zero_shapes = [], [], [], []
    for alloc in nc.m.functions[0].allocations:
        if not isinstance(alloc, mybir.MemoryLocationSet):
            continue
        name = alloc.memorylocations[0].name
        if alloc.kind == "ExternalInput":
            if name != partition_name:
                in_names.append(name)
        elif alloc.kind == "ExternalOutput":
            out_names.append(name)
            shape = tuple(alloc.tensor_shape)
            dtype = mybir.dt.np(alloc.dtype)
            out_avals.append(jax.core.ShapedArray(shape, dtype))
            zero_shapes.append((shape, dtype))
    n_params = len(in_names)
    all_in_names = list(in_names) + list(out_names)
    if partition_name is not None:
        all_in_names.append(partition_name)

    def _body(*args):
        operands = list(args)
        if partition_name is not None:
            operands.append(bass2jax.partition_id_tensor())
        outs = bass2jax._bass_exec_p.bind(
            *operands,
            out_avals=tuple(out_avals),
            in_names=tuple(all_in_names),
            out_names=tuple(out_names),
            lowering_input_output_aliases=(),
            sim_require_finite=True,
            sim_require_nnan=True,
            nc=nc,
        )
        return tuple(outs)

    devices = jax.devices()[:N_CORES]
    assert len(devices) == N_CORES, f"need {N_CORES} devices"
    mesh = Mesh(np.asarray(devices), ("core",))
    n_outs = len(out_names)
    in_specs = (PartitionSpec("core"),) * (n_params + n_outs)
    out_specs = (PartitionSpec("core"),) * n_outs
    donate = tuple(range(n_params, n_params + n_outs))
    sharded = jax.jit(
        shard_map(_body, mesh=mesh, in_specs=in_specs, out_specs=out_specs,
                  check_rep=False),
        donate_argnums=donate, keep_unused=True)

    def make_zeros():
        return [jnp.zeros((N_CORES * s[0], *s[1:]), d)
                for (s, d) in zero_shapes]

    ex = {"nc": nc, "sharded": sharded, "in_names": in_names,
          "out_names": out_names, "make_zeros": make_zeros,
          "n_params": n_params}
    _prog_cache["exec"] = ex
    return ex


def _concat_inputs(x, w8, w16, w32):
    """Global (8*R, ...) concat inputs keyed for the program."""
    x = np.asarray(x, np.float32).reshape(B_FULL, T_FULL)
    wband = _build_wband(w8, w16, w32)
    ident = np.ascontiguousarray(np.eye(128, dtype=np.float32))
    per = {
        "x": x,                                       # already (8*R, T)
        "wband": np.concatenate([wband] * N_CORES, axis=0),
        "ident": np.concatenate([ident] * N_CORES, axis=0),
    }
    ex = _get_exec()
    return [per[name] for name in ex["in_names"]]


def kernel(x, y=None, w8=None, w16=None, w32=None):
    """Full-input entry point: x [256,1,32768], returns (u, s_all)."""
    ex = _get_exec()
    concat_in = _concat_inputs(x, w8, w16, w32)
    outs = ex["sharded"](*concat_in, *ex["make_zeros"]())
    res = {name: np.asarray(outs[i]) for i, name in enumerate(ex["out_names"])}
    u = res["u"].reshape(B_FULL, KCH, T_FULL)
    s = res["s"].reshape(B_FULL, KCH, T_FULL)
    return u, s


def bench(x, w8, w16, w32, iters=10):
    """Return list of per-call wall times (s) with device-resident I/O."""
    import time as _time
    import jax
    from jax.sharding import Mesh, PartitionSpec, NamedSharding
    ex = _get_exec()
    concat_in = _concat_inputs(x, w8, w16, w32)
    mesh = Mesh(np.asarray(jax.devices()[:N_CORES]), ("core",))
    sh = NamedSharding(mesh, PartitionSpec("core"))
    dev_in = [jax.device_put(a, sh) for a in concat_in]
    jax.block_until_ready(ex["sharded"](*dev_in, *ex["make_zeros"]()))
    times = []
    for _ in range(iters):
        zeros = ex["make_zeros"]()
        jax.block_until_ready(zeros)
        t0 = _time.perf_counter()
        outs = ex["sharded"](*dev_in, *zeros)
        jax.block_until_ready(outs)
        times.append(_time.perf_counter() - t0)
    return times
